# revision 4
# baseline (speedup 1.0000x reference)
"""CondConv (MoE routed conv) Trainium2 Bass kernel.

Strategy (8 NeuronCores, data-parallel over batch, 2 samples/core):
  1. Routing on device: GAP via DVE reduce, linear via PE matmul, sigmoid on ACT.
  2. Per-sample combined conv weights cw[s] = sum_e r[s,e] * W[e] on DVE
     (scalar_tensor_tensor fused multiply-accumulate), accumulated in fp32,
     final expert writes the float32r copy consumed by the PE.
  3. 3x3 conv as 18 accumulating PE matmuls per output tile (2 cin K-tiles x
     9 taps), float32r (1 cycle/row at N>=256, ~1.5e-4 rel err), zero-padding
     realized via clipped access patterns + PSUM has_written semantics.
  4. BN (inference) + SiLU fused into one ACT activation per output tile
     (scale/bias are per-partition vectors folded on host).
"""

import sys

sys.path.insert(0, "/opt/trn_rl_repo")

import numpy as np

import concourse.bass as bass  # noqa: F401  (engine registration side effects)
import concourse.mybir as mybir
import concourse.tile as tile
from concourse import bacc
from concourse.bass_utils import run_bass_kernel_spmd

F32 = mybir.dt.float32
F32R = mybir.dt.float32r
AF = mybir.ActivationFunctionType
ALU = mybir.AluOpType

B, CIN, H, W = 16, 256, 56, 56
E, COUT, KS = 8, 256, 3
NCORES = 8
SPC = B // NCORES  # samples per core
IT = CIN // 128  # cin partition tiles
OT = COUT // 128  # cout partition tiles
KHKW = KS * KS
HB = 9  # rows per h-block -> N = 9*56 = 504 <= 512 (one PSUM bank)
WP = W + 2  # host-padded width (zero cols at w=0 and w=57)
PIX = H * W
BN_EPS = 1e-5

_PROGRAM_CACHE = {}


def _build_program():
    nc = bacc.Bacc("TRN2", target_bir_lowering=False, debug=False)

    x_d = nc.dram_tensor("x", [SPC, IT, 128, H, WP], F32R, kind="ExternalInput")
    wt_d = nc.dram_tensor("wt", [E, IT, 128, KHKW * COUT], F32, kind="ExternalInput")
    rwt_d = nc.dram_tensor("rwt", [IT, 128, E], F32, kind="ExternalInput")
    rb_d = nc.dram_tensor("rb", [1, E], F32, kind="ExternalInput")
    bns_d = nc.dram_tensor("bns", [OT, 128, 1], F32, kind="ExternalInput")
    bnb_d = nc.dram_tensor("bnb", [OT, 128, 1], F32, kind="ExternalInput")
    y_d = nc.dram_tensor("y", [SPC, OT, 128, H, W], F32, kind="ExternalOutput")

    with tile.TileContext(nc) as tc:
        with (
            tc.tile_pool(name="xp", bufs=1) as xp,
            tc.tile_pool(name="cwp", bufs=1) as cwp,
            tc.tile_pool(name="wtp", bufs=3) as wtp,
            tc.tile_pool(name="outp", bufs=4) as outp,
            tc.tile_pool(name="smal", bufs=1) as smal,
            tc.tile_pool(name="psc", bufs=5, space="PSUM") as psc,
            tc.tile_pool(name="pss", bufs=1, space="PSUM") as pss,
        ):
            # ---- input loads ----
            x_sb = {}
            for s in range(SPC):
                for it in range(IT):
                    t = xp.tile([128, H, WP], F32R, tag=f"x_{s}_{it}")
                    nc.sync.dma_start(t[:], x_d[s, it])
                    x_sb[s, it] = t

            rwt_sb = []
            for it in range(IT):
                t = smal.tile([128, E], F32, tag=f"rwt{it}")
                nc.gpsimd.dma_start(t[:], rwt_d[it])
                rwt_sb.append(t)
            rb_sb = smal.tile([1, E], F32, tag="rb")
            nc.gpsimd.dma_start(rb_sb[:], rb_d[:])
            bns_sb, bnb_sb = [], []
            for ot in range(OT):
                ts_ = smal.tile([128, 1], F32, tag=f"bns{ot}")
                nc.gpsimd.dma_start(ts_[:], bns_d[ot])
                bns_sb.append(ts_)
                tb_ = smal.tile([128, 1], F32, tag=f"bnb{ot}")
                nc.gpsimd.dma_start(tb_[:], bnb_d[ot])
                bnb_sb.append(tb_)
            ones_sb = smal.tile([1, 128], F32, tag="ones")
            nc.vector.memset(ones_sb[:], 1.0)

            # ---- routing: r[s] = sigmoid(mean(x) @ rwT + rb), broadcast to 128 partitions ----
            r_bcast = []
            for s in range(SPC):
                pooled = []
                for it in range(IT):
                    p = smal.tile([128, 1], F32, tag=f"pool{s}{it}")
                    nc.vector.reduce_sum(
                        p[:],
                        x_sb[s, it][:].rearrange("p a b -> p (a b)"),
                        axis=mybir.AxisListType.X,
                    )
                    pooled.append(p)
                lg_ps = pss.tile([1, E], F32, tag="rps")
                for it in range(IT):
                    nc.tensor.matmul(
                        lg_ps[:], pooled[it][:], rwt_sb[it][:],
                        start=(it == 0), stop=(it == IT - 1),
                    )
                zrow = smal.tile([1, E], F32, tag=f"z{s}")
                nc.vector.scalar_tensor_tensor(
                    zrow[:], lg_ps[:], 1.0 / PIX, rb_sb[:], ALU.mult, ALU.add
                )
                rrow = smal.tile([1, E], F32, tag=f"r{s}")
                nc.scalar.activation(rrow[:], zrow[:], AF.Sigmoid)
                rb_ps = pss.tile([128, E], F32, tag="rps2")
                nc.tensor.matmul(rb_ps[:], ones_sb[:], rrow[:], start=True, stop=True)
                rbc = smal.tile([128, E], F32, tag=f"rbc{s}")
                nc.vector.tensor_copy(rbc[:], rb_ps[:])
                r_bcast.append(rbc)

            # ---- combine: cw[s][it] = sum_e r[s,e] * wt[e,it] ----
            cw_f32 = {
                (s, it): cwp.tile(
                    [128, KHKW * COUT], F32, tag=f"cw_{s}_{it}", name=f"cw_{s}_{it}"
                )
                for s in range(SPC)
                for it in range(IT)
            }
            cw_r = {
                (s, it): cwp.tile(
                    [128, KHKW * COUT], F32R, tag=f"cwr_{s}_{it}", name=f"cwr_{s}_{it}"
                )
                for s in range(SPC)
                for it in range(IT)
            }
            for e in range(E):
                for it in range(IT):
                    wt_t = wtp.tile([128, KHKW * COUT], F32, tag="wt")
                    nc.scalar.dma_start(wt_t[:], wt_d[e, it])
                    for s in range(SPC):
                        sc = r_bcast[s][:, e : e + 1]
                        if e == 0:
                            nc.vector.tensor_scalar_mul(cw_f32[s, it][:], wt_t[:], sc)
                        elif e < E - 1:
                            nc.vector.scalar_tensor_tensor(
                                cw_f32[s, it][:], wt_t[:], sc, cw_f32[s, it][:],
                                ALU.mult, ALU.add,
                            )
                        else:
                            nc.vector.scalar_tensor_tensor(
                                cw_r[s, it][:], wt_t[:], sc, cw_f32[s, it][:],
                                ALU.mult, ALU.add,
                            )

            # ---- conv (18 accumulating matmuls / tile) + BN + SiLU epilogue ----
            hblocks = [(h0, min(HB, H - h0)) for h0 in range(0, H, HB)]
            # center tap first: full coverage -> start=True initializes the bank
            taps = [(0, 0)] + [
                (dh, dw) for dh in (-1, 0, 1) for dw in (-1, 0, 1) if (dh, dw) != (0, 0)
            ]
            for s in range(SPC):
                for ot in range(OT):
                    for h0, nh in hblocks:
                        ps_t = psc.tile([128, HB, W], F32, tag="ps")
                        n_mm = 0
                        total = IT * sum(
                            1 for dh, dw in taps if min(h0 + nh, H - dh) > max(h0, -dh)
                        )
                        for dh, dw in taps:
                            khkw = (dh + 1) * 3 + (dw + 1)
                            ho_s = max(h0, -dh)
                            ho_e = min(h0 + nh, H - dh)
                            if ho_e <= ho_s:
                                continue
                            nhh = ho_e - ho_s
                            hi_s = ho_s + dh
                            for it in range(IT):
                                off = khkw * COUT + ot * 128
                                lhsT = cw_r[s, it][:, off : off + 128]
                                rhs = x_sb[s, it][:, hi_s : hi_s + nhh, 1 + dw : 1 + dw + W]
                                out = ps_t[:, ho_s - h0 : ho_s - h0 + nhh, 0:W]
                                nc.tensor.matmul(
                                    out, lhsT, rhs,
                                    start=(n_mm == 0), stop=(n_mm == total - 1),
                                )
                                n_mm += 1
                        o_t = outp.tile([128, HB, W], F32, tag="out")
                        nc.scalar.activation(
                            o_t[:, :nh, :], ps_t[:, :nh, :], AF.Silu,
                            bias=bnb_sb[ot][:], scale=bns_sb[ot][:],
                        )
                        nc.gpsimd.dma_start(y_d[s, ot, :, h0 : h0 + nh, :], o_t[:, :nh, :])

    nc.compile()
    return nc


def _get_program():
    if "nc" not in _PROGRAM_CACHE:
        _PROGRAM_CACHE["nc"] = _build_program()
    return _PROGRAM_CACHE["nc"]


def kernel(x, routing_w, routing_b, kernel_weights, bn_gamma, bn_beta, bn_mean, bn_var,
           _trace=False, _trace_kwargs=None):
    x = np.asarray(x, dtype=np.float32)
    routing_w = np.asarray(routing_w, dtype=np.float32)
    routing_b = np.asarray(routing_b, dtype=np.float32)
    kernel_weights = np.asarray(kernel_weights, dtype=np.float32)
    bn_gamma = np.asarray(bn_gamma, dtype=np.float32)
    bn_beta = np.asarray(bn_beta, dtype=np.float32)
    bn_mean = np.asarray(bn_mean, dtype=np.float32)
    bn_var = np.asarray(bn_var, dtype=np.float32)

    # host-side layout prep
    # wt[e, it, i, khkw*COUT + o] from kernel_weights[e, o, i, kh, kw]
    wt_host = np.ascontiguousarray(kernel_weights.transpose(0, 2, 3, 4, 1)).reshape(
        E, IT, 128, KHKW * COUT
    )
    rwt_host = np.ascontiguousarray(routing_w.T).reshape(IT, 128, E)
    rb_host = np.ascontiguousarray(routing_b).reshape(1, E)
    inv = bn_gamma / np.sqrt(bn_var + BN_EPS)
    bns_host = np.ascontiguousarray(inv).reshape(OT, 128, 1)
    bnb_host = np.ascontiguousarray(bn_beta - bn_mean * inv).reshape(OT, 128, 1)

    x_pad = np.zeros((B, CIN, H, WP), dtype=np.float32)
    x_pad[:, :, :, 1 : 1 + W] = x
    in_maps = []
    for g in range(NCORES):
        xg = np.ascontiguousarray(
            x_pad[g * SPC : (g + 1) * SPC].reshape(SPC, IT, 128, H, WP)
        )
        in_maps.append(
            {
                "x": xg,
                "wt": wt_host,
                "rwt": rwt_host,
                "rb": rb_host,
                "bns": bns_host,
                "bnb": bnb_host,
            }
        )

    nc = _get_program()
    res = run_bass_kernel_spmd(
        nc, in_maps, core_ids=list(range(NCORES)),
        trace=_trace, **(_trace_kwargs or {}),
    )
    _PROGRAM_CACHE["last_result"] = res

    out = np.empty((B, COUT, H, W), dtype=np.float32)
    for g in range(NCORES):
        yg = res.results[g]["y"]  # [SPC, OT, 128, H, W]
        out[g * SPC : (g + 1) * SPC] = yg.reshape(SPC, COUT, H, W)
    return out


# revision 6
# speedup vs baseline: 1.0591x; 1.0591x over previous
"""CondConv (MoE routed conv) Trainium2 Bass kernel.

Strategy (8 NeuronCores, data-parallel over batch, 2 samples/core):
  1. Routing on device: GAP via DVE reduce, linear via PE matmul, sigmoid on ACT.
  2. Per-sample combined conv weights cw[s] = sum_e r[s,e] * W[e] computed on
     the PE via a diagonal trick: cw_chunk = sum_e (r[s,e]*I).T @ W[e]_chunk,
     accumulated exactly in PSUM fp32, single float32r rounding on the ACT
     PSUM->SBUF copy.
  3. cw is laid out cout-half-major so the conv for output half ot=0 only
     needs the first half of the weight stream -> the DMA fill is halved.
  4. 3x3 conv as 18 accumulating PE matmuls per output tile (2 cin K-tiles x
     9 taps), float32r (1 cycle/row at N>=256, ~1.5e-4 rel err), zero padding
     via host-padded x width (58) and h-clipped access patterns + PSUM
     has_written semantics.
  5. BN (inference) + SiLU fused into one ACT activation per output tile
     (scale/bias per-partition vectors folded on host).
"""

import sys

sys.path.insert(0, "/opt/trn_rl_repo")

import numpy as np

import concourse.bass as bass  # noqa: F401
import concourse.mybir as mybir
import concourse.tile as tile
from concourse import bacc
from concourse.bass_utils import run_bass_kernel_spmd

F32 = mybir.dt.float32
F32R = mybir.dt.float32r
AF = mybir.ActivationFunctionType
ALU = mybir.AluOpType

B, CIN, H, W = 16, 256, 56, 56
E, COUT, KS = 8, 256, 3
NCORES = 8
SPC = B // NCORES  # samples per core
IT = CIN // 128  # cin partition tiles
OT = COUT // 128  # cout partition tiles
KHKW = KS * KS
HB = 9  # rows per h-block -> N = 9*56 = 504 <= 512 (one PSUM bank)
WP = W + 2  # host-padded width (zero cols at w=0 and w=57)
PIX = H * W
BN_EPS = 1e-5
SLAB = KHKW * 128  # 1152 cw columns per (it, ot)
CHUNK = 384  # combine psum chunk (3 chunks per slab, >=256 keeps f32r rate)
NCH = SLAB // CHUNK

_PROGRAM_CACHE = {}


def _build_program():
    nc = bacc.Bacc("TRN2", target_bir_lowering=False, debug=False)

    x_d = nc.dram_tensor("x", [SPC, IT, 128, H, WP], F32R, kind="ExternalInput")
    # wt[e, ot, it, i, khkw*128 + o_in]  (slab-major for streaming by ot)
    wt_d = nc.dram_tensor("wt", [E, OT, IT, 128, SLAB], F32R, kind="ExternalInput")
    rwt_d = nc.dram_tensor("rwt", [IT, 128, E], F32, kind="ExternalInput")
    rb_d = nc.dram_tensor("rb", [1, E], F32, kind="ExternalInput")
    ident_d = nc.dram_tensor("ident", [128, 128], F32, kind="ExternalInput")
    bns_d = nc.dram_tensor("bns", [OT, 128, 1], F32, kind="ExternalInput")
    bnb_d = nc.dram_tensor("bnb", [OT, 128, 1], F32, kind="ExternalInput")
    y_d = nc.dram_tensor("y", [SPC, OT, 128, H, W], F32, kind="ExternalOutput")

    with tile.TileContext(nc) as tc:
        with (
            tc.tile_pool(name="xp", bufs=1) as xp,
            tc.tile_pool(name="cwp", bufs=1) as cwp,
            tc.tile_pool(name="wtp", bufs=10) as wtp,
            tc.tile_pool(name="outp", bufs=4) as outp,
            tc.tile_pool(name="smal", bufs=1) as smal,
            tc.tile_pool(name="psc", bufs=5, space="PSUM") as psc,
            tc.tile_pool(name="psk", bufs=2, space="PSUM") as psk,
            tc.tile_pool(name="pss", bufs=1, space="PSUM") as pss,
        ):
            # ---- input loads ----
            x_sb = {}
            for s in range(SPC):
                for it in range(IT):
                    t = xp.tile([128, H, WP], F32R, tag=f"x_{s}_{it}", name=f"x_{s}_{it}")
                    nc.sync.dma_start(t[:], x_d[s, it])
                    x_sb[s, it] = t

            rwt_sb = []
            for it in range(IT):
                t = smal.tile([128, E], F32, tag=f"rwt{it}", name=f"rwt{it}")
                nc.gpsimd.dma_start(t[:], rwt_d[it])
                rwt_sb.append(t)
            rb_sb = smal.tile([1, E], F32, tag="rb")
            nc.gpsimd.dma_start(rb_sb[:], rb_d[:])
            ident_sb = smal.tile([128, 128], F32, tag="ident")
            nc.gpsimd.dma_start(ident_sb[:], ident_d[:])
            bns_sb, bnb_sb = [], []
            for ot in range(OT):
                ts_ = smal.tile([128, 1], F32, tag=f"bns{ot}", name=f"bns{ot}")
                nc.gpsimd.dma_start(ts_[:], bns_d[ot])
                bns_sb.append(ts_)
                tb_ = smal.tile([128, 1], F32, tag=f"bnb{ot}", name=f"bnb{ot}")
                nc.gpsimd.dma_start(tb_[:], bnb_d[ot])
                bnb_sb.append(tb_)
            ones_sb = smal.tile([1, 128], F32, tag="ones")
            nc.vector.memset(ones_sb[:], 1.0)

            # ---- routing: r[s] = sigmoid(mean(x) @ rwT + rb) -> bcast [128, E] ----
            r_bcast = []
            for s in range(SPC):
                pooled = []
                for it in range(IT):
                    p = smal.tile([128, 1], F32, tag=f"pool{s}{it}", name=f"pool{s}{it}")
                    nc.vector.reduce_sum(
                        p[:],
                        x_sb[s, it][:].rearrange("p a b -> p (a b)"),
                        axis=mybir.AxisListType.X,
                    )
                    pooled.append(p)
                lg_ps = pss.tile([1, E], F32, tag="rps", name=f"lgps{s}")
                for it in range(IT):
                    nc.tensor.matmul(
                        lg_ps[:], pooled[it][:], rwt_sb[it][:],
                        start=(it == 0), stop=(it == IT - 1),
                    )
                zrow = smal.tile([1, E], F32, tag=f"z{s}", name=f"z{s}")
                nc.vector.scalar_tensor_tensor(
                    zrow[:], lg_ps[:], 1.0 / PIX, rb_sb[:], ALU.mult, ALU.add
                )
                rrow = smal.tile([1, E], F32, tag=f"r{s}", name=f"r{s}")
                nc.scalar.activation(rrow[:], zrow[:], AF.Sigmoid)
                rb_ps = pss.tile([128, E], F32, tag="rps", name=f"rbps{s}")
                nc.tensor.matmul(rb_ps[:], ones_sb[:], rrow[:], start=True, stop=True)
                rbc = smal.tile([128, E], F32, tag=f"rbc{s}", name=f"rbc{s}")
                nc.vector.tensor_copy(rbc[:], rb_ps[:])
                r_bcast.append(rbc)

            # diag[s,e] = r[s,e] * I, rounded to f32r by the ACT copy
            diag = {}
            for s in range(SPC):
                for e in range(E):
                    dt_ = smal.tile(
                        [128, 128], F32R, tag=f"diag{s}{e}", name=f"diag{s}{e}"
                    )
                    nc.scalar.activation(
                        dt_[:], ident_sb[:], AF.Copy, scale=r_bcast[s][:, e : e + 1]
                    )
                    diag[s, e] = dt_

            # persistent cw tiles (f32r), slab layout [i, khkw*128 + o_in]
            cw_r = {
                (s, it, ot): cwp.tile(
                    [128, SLAB], F32R,
                    tag=f"cwr_{s}_{it}_{ot}", name=f"cwr_{s}_{it}_{ot}",
                )
                for s in range(SPC)
                for it in range(IT)
                for ot in range(OT)
            }

            hblocks = [(h0, min(HB, H - h0)) for h0 in range(0, H, HB)]
            taps = [(0, 0)] + [
                (dh, dw) for dh in (-1, 0, 1) for dw in (-1, 0, 1) if (dh, dw) != (0, 0)
            ]

            def combine(ot):
                # stream wt slabs for this ot, accumulate cw on PE via diag trick
                for it in range(IT):
                    slabs = []
                    for e in range(E):
                        wt_t = wtp.tile([128, SLAB], F32R, tag="wt", name=f"wt{ot}{it}{e}")
                        nc.scalar.dma_start(wt_t[:], wt_d[e, ot, it])
                        slabs.append(wt_t)
                    for c in range(NCH):
                        for s in range(SPC):
                            kps = psk.tile([128, CHUNK], F32, tag="kps", name="kps")
                            for e in range(E):
                                nc.tensor.matmul(
                                    kps[:],
                                    diag[s, e][:],
                                    slabs[e][:, c * CHUNK : (c + 1) * CHUNK],
                                    start=(e == 0),
                                    stop=(e == E - 1),
                                )
                            nc.scalar.activation(
                                cw_r[s, it, ot][:, c * CHUNK : (c + 1) * CHUNK],
                                kps[:],
                                AF.Copy,
                            )

            def conv(ot):
                for s in range(SPC):
                    for h0, nh in hblocks:
                        ps_t = psc.tile([128, HB, W], F32, tag="ps", name="ps")
                        n_mm = 0
                        total = IT * sum(
                            1 for dh, dw in taps if min(h0 + nh, H - dh) > max(h0, -dh)
                        )
                        for dh, dw in taps:
                            khkw = (dh + 1) * 3 + (dw + 1)
                            ho_s = max(h0, -dh)
                            ho_e = min(h0 + nh, H - dh)
                            if ho_e <= ho_s:
                                continue
                            nhh = ho_e - ho_s
                            hi_s = ho_s + dh
                            for it in range(IT):
                                off = khkw * 128
                                lhsT = cw_r[s, it, ot][:, off : off + 128]
                                rhs = x_sb[s, it][:, hi_s : hi_s + nhh, 1 + dw : 1 + dw + W]
                                out = ps_t[:, ho_s - h0 : ho_s - h0 + nhh, 0:W]
                                nc.tensor.matmul(
                                    out, lhsT, rhs,
                                    start=(n_mm == 0), stop=(n_mm == total - 1),
                                )
                                n_mm += 1
                        o_t = outp.tile([128, HB, W], F32, tag="out", name="o_t")
                        nc.scalar.activation(
                            o_t[:, :nh, :], ps_t[:, :nh, :], AF.Silu,
                            bias=bnb_sb[ot][:], scale=bns_sb[ot][:],
                        )
                        nc.gpsimd.dma_start(
                            y_d[s, ot, :, h0 : h0 + nh, :], o_t[:, :nh, :]
                        )

            for ot in range(OT):
                combine(ot)
                conv(ot)

    nc.compile()
    return nc


def _get_program():
    if "nc" not in _PROGRAM_CACHE:
        _PROGRAM_CACHE["nc"] = _build_program()
    return _PROGRAM_CACHE["nc"]


def kernel(x, routing_w, routing_b, kernel_weights, bn_gamma, bn_beta, bn_mean, bn_var,
           _trace=False, _trace_kwargs=None):
    x = np.asarray(x, dtype=np.float32)
    routing_w = np.asarray(routing_w, dtype=np.float32)
    routing_b = np.asarray(routing_b, dtype=np.float32)
    kernel_weights = np.asarray(kernel_weights, dtype=np.float32)
    bn_gamma = np.asarray(bn_gamma, dtype=np.float32)
    bn_beta = np.asarray(bn_beta, dtype=np.float32)
    bn_mean = np.asarray(bn_mean, dtype=np.float32)
    bn_var = np.asarray(bn_var, dtype=np.float32)

    # wt[e, ot, it, i, khkw*128 + o_in] from kernel_weights[e, o, i, kh, kw]
    # o = ot*128 + o_in ; khkw = kh*3 + kw
    kw7 = kernel_weights.reshape(E, OT, 128, IT, 128, KS, KS)
    wt_host = np.ascontiguousarray(kw7.transpose(0, 1, 3, 4, 5, 6, 2)).reshape(
        E, OT, IT, 128, SLAB
    )
    rwt_host = np.ascontiguousarray(routing_w.T).reshape(IT, 128, E)
    rb_host = np.ascontiguousarray(routing_b).reshape(1, E)
    ident_host = np.eye(128, dtype=np.float32)
    inv = bn_gamma / np.sqrt(bn_var + BN_EPS)
    bns_host = np.ascontiguousarray(inv).reshape(OT, 128, 1)
    bnb_host = np.ascontiguousarray(bn_beta - bn_mean * inv).reshape(OT, 128, 1)

    x_pad = np.zeros((B, CIN, H, WP), dtype=np.float32)
    x_pad[:, :, :, 1 : 1 + W] = x
    in_maps = []
    for g in range(NCORES):
        xg = np.ascontiguousarray(
            x_pad[g * SPC : (g + 1) * SPC].reshape(SPC, IT, 128, H, WP)
        )
        in_maps.append(
            {
                "x": xg,
                "wt": wt_host,
                "rwt": rwt_host,
                "rb": rb_host,
                "ident": ident_host,
                "bns": bns_host,
                "bnb": bnb_host,
            }
        )

    nc = _get_program()
    res = run_bass_kernel_spmd(
        nc, in_maps, core_ids=list(range(NCORES)),
        trace=_trace, **(_trace_kwargs or {}),
    )
    _PROGRAM_CACHE["last_result"] = res

    out = np.empty((B, COUT, H, W), dtype=np.float32)
    for g in range(NCORES):
        yg = res.results[g]["y"]  # [SPC, OT, 128, H, W]
        out[g * SPC : (g + 1) * SPC] = yg.reshape(SPC, COUT, H, W)
    return out


# revision 7
# speedup vs baseline: 1.2816x; 1.2100x over previous
"""CondConv (MoE routed conv) Trainium2 Bass kernel.

Strategy (8 NeuronCores, data-parallel over batch, 2 samples/core):
  1. Routing on device: GAP via DVE reduce, linear via PE matmul, sigmoid on ACT.
  2. Per-sample combined conv weights cw[s] = sum_e r[s,e] * W[e] computed on
     the PE via a diagonal trick: cw_chunk = sum_e (r[s,e]*I).T @ W[e]_chunk,
     accumulated exactly in PSUM fp32, single float32r rounding on the ACT
     PSUM->SBUF copy.
  3. cw is laid out cout-half-major so the conv for output half ot=0 only
     needs the first half of the weight stream -> the DMA fill is halved.
  4. 3x3 conv as 18 accumulating PE matmuls per output tile (2 cin K-tiles x
     9 taps), float32r (1 cycle/row at N>=256, ~1.5e-4 rel err), zero padding
     via host-padded x width (58) and h-clipped access patterns + PSUM
     has_written semantics.
  5. BN (inference) + SiLU fused into one ACT activation per output tile
     (scale/bias per-partition vectors folded on host).
"""

import sys

sys.path.insert(0, "/opt/trn_rl_repo")

import numpy as np

import concourse.bass as bass  # noqa: F401
import concourse.mybir as mybir
import concourse.tile as tile
from concourse import bacc
from concourse.bass_utils import run_bass_kernel_spmd

F32 = mybir.dt.float32
F32R = mybir.dt.float32r
AF = mybir.ActivationFunctionType
ALU = mybir.AluOpType

B, CIN, H, W = 16, 256, 56, 56
E, COUT, KS = 8, 256, 3
NCORES = 8
SPC = B // NCORES  # samples per core
IT = CIN // 128  # cin partition tiles
OT = COUT // 128  # cout partition tiles
KHKW = KS * KS
HB = 9  # rows per h-block -> N = 9*56 = 504 <= 512 (one PSUM bank)
WP = W + 2  # host-padded width (zero cols at w=0 and w=57)
PIX = H * W
BN_EPS = 1e-5
SLAB = KHKW * 128  # 1152 cw columns per (it, ot)
CHUNK = 384  # combine psum chunk (3 chunks per slab, >=256 keeps f32r rate)
NCH = SLAB // CHUNK

_PROGRAM_CACHE = {}


def _build_program():
    nc = bacc.Bacc("TRN2", target_bir_lowering=False, debug=False)

    x_d = nc.dram_tensor("x", [SPC, IT, 128, H, WP], F32R, kind="ExternalInput")
    # wt[e, ot, it, i, khkw*128 + o_in]  (slab-major for streaming by ot)
    wt_d = nc.dram_tensor("wt", [E, OT, IT, 128, SLAB], F32R, kind="ExternalInput")
    rwt_d = nc.dram_tensor("rwt", [IT, 128, E], F32, kind="ExternalInput")
    rb_d = nc.dram_tensor("rb", [1, E], F32, kind="ExternalInput")
    ident_d = nc.dram_tensor("ident", [128, 128], F32, kind="ExternalInput")
    bns_d = nc.dram_tensor("bns", [OT, 128, 1], F32, kind="ExternalInput")
    bnb_d = nc.dram_tensor("bnb", [OT, 128, 1], F32, kind="ExternalInput")
    y_d = nc.dram_tensor("y", [SPC, OT, 128, H, W], F32, kind="ExternalOutput")

    with tile.TileContext(nc) as tc:
        with (
            tc.tile_pool(name="xp", bufs=1) as xp,
            tc.tile_pool(name="cwp", bufs=1) as cwp,
            tc.tile_pool(name="wtp", bufs=10) as wtp,
            tc.tile_pool(name="outp", bufs=4) as outp,
            tc.tile_pool(name="smal", bufs=1) as smal,
            tc.tile_pool(name="psc", bufs=5, space="PSUM") as psc,
            tc.tile_pool(name="psk", bufs=2, space="PSUM") as psk,
            tc.tile_pool(name="pss", bufs=1, space="PSUM") as pss,
        ):
            # ---- input loads ----
            x_sb = {}
            for s in range(SPC):
                for it in range(IT):
                    t = xp.tile([128, H, WP], F32R, tag=f"x_{s}_{it}", name=f"x_{s}_{it}")
                    nc.sync.dma_start(t[:], x_d[s, it])
                    x_sb[s, it] = t

            rwt_sb = []
            for it in range(IT):
                t = smal.tile([128, E], F32, tag=f"rwt{it}", name=f"rwt{it}")
                nc.gpsimd.dma_start(t[:], rwt_d[it])
                rwt_sb.append(t)
            rb_sb = smal.tile([1, E], F32, tag="rb")
            nc.gpsimd.dma_start(rb_sb[:], rb_d[:])
            ident_sb = smal.tile([128, 128], F32, tag="ident")
            nc.gpsimd.dma_start(ident_sb[:], ident_d[:])
            bns_sb, bnb_sb = [], []
            for ot in range(OT):
                ts_ = smal.tile([128, 1], F32, tag=f"bns{ot}", name=f"bns{ot}")
                nc.gpsimd.dma_start(ts_[:], bns_d[ot])
                bns_sb.append(ts_)
                tb_ = smal.tile([128, 1], F32, tag=f"bnb{ot}", name=f"bnb{ot}")
                nc.gpsimd.dma_start(tb_[:], bnb_d[ot])
                bnb_sb.append(tb_)
            ones_sb = smal.tile([1, 128], F32, tag="ones")
            nc.vector.memset(ones_sb[:], 1.0)

            # ---- routing: r[s] = sigmoid(mean(x) @ rwT + rb) -> bcast [128, E] ----
            r_bcast = []
            for s in range(SPC):
                pooled = []
                for it in range(IT):
                    p = smal.tile([128, 1], F32, tag=f"pool{s}{it}", name=f"pool{s}{it}")
                    nc.vector.reduce_sum(
                        p[:],
                        x_sb[s, it][:].rearrange("p a b -> p (a b)"),
                        axis=mybir.AxisListType.X,
                    )
                    pooled.append(p)
                lg_ps = pss.tile([1, E], F32, tag="rps", name=f"lgps{s}")
                for it in range(IT):
                    nc.tensor.matmul(
                        lg_ps[:], pooled[it][:], rwt_sb[it][:],
                        start=(it == 0), stop=(it == IT - 1),
                    )
                zrow = smal.tile([1, E], F32, tag=f"z{s}", name=f"z{s}")
                nc.vector.scalar_tensor_tensor(
                    zrow[:], lg_ps[:], 1.0 / PIX, rb_sb[:], ALU.mult, ALU.add
                )
                rrow = smal.tile([1, E], F32, tag=f"r{s}", name=f"r{s}")
                nc.scalar.activation(rrow[:], zrow[:], AF.Sigmoid)
                rb_ps = pss.tile([128, E], F32, tag="rps", name=f"rbps{s}")
                nc.tensor.matmul(rb_ps[:], ones_sb[:], rrow[:], start=True, stop=True)
                rbc = smal.tile([128, E], F32, tag=f"rbc{s}", name=f"rbc{s}")
                nc.vector.tensor_copy(rbc[:], rb_ps[:])
                r_bcast.append(rbc)

            # diag[s,e] = r[s,e] * I, rounded to f32r by the ACT copy
            diag = {}
            for s in range(SPC):
                for e in range(E):
                    dt_ = smal.tile(
                        [128, 128], F32R, tag=f"diag{s}{e}", name=f"diag{s}{e}"
                    )
                    nc.scalar.activation(
                        dt_[:], ident_sb[:], AF.Copy, scale=r_bcast[s][:, e : e + 1]
                    )
                    diag[s, e] = dt_

            # persistent cw tiles (f32r), slab layout [i, khkw*128 + o_in]
            cw_r = {
                (s, it, ot): cwp.tile(
                    [128, SLAB], F32R,
                    tag=f"cwr_{s}_{it}_{ot}", name=f"cwr_{s}_{it}_{ot}",
                )
                for s in range(SPC)
                for it in range(IT)
                for ot in range(OT)
            }

            hblocks = [(h0, min(HB, H - h0)) for h0 in range(0, H, HB)]
            taps = [(0, 0)] + [
                (dh, dw) for dh in (-1, 0, 1) for dw in (-1, 0, 1) if (dh, dw) != (0, 0)
            ]

            # prefetch all wt slabs in consumption order on the sync ring
            # (x DMAs are ahead of them in the same FIFO; pool slots gate issue)
            slab_tiles = {}
            for ot in range(OT):
                for it in range(IT):
                    for e in range(E):
                        wt_t = wtp.tile(
                            [128, SLAB], F32R, tag="wt", name=f"wt{ot}{it}{e}"
                        )
                        nc.sync.dma_start(wt_t[:], wt_d[e, ot, it])
                        slab_tiles[ot, it, e] = wt_t

            def combine_pe(ot):
                # accumulate cw on PE via diag trick
                for it in range(IT):
                    slabs = [slab_tiles[ot, it, e] for e in range(E)]
                    for c in range(NCH):
                        for s in range(SPC):
                            kps = psk.tile([128, CHUNK], F32, tag="kps", name="kps")
                            for e in range(E):
                                nc.tensor.matmul(
                                    kps[:],
                                    diag[s, e][:],
                                    slabs[e][:, c * CHUNK : (c + 1) * CHUNK],
                                    start=(e == 0),
                                    stop=(e == E - 1),
                                )
                            nc.scalar.activation(
                                cw_r[s, it, ot][:, c * CHUNK : (c + 1) * CHUNK],
                                kps[:],
                                AF.Copy,
                            )

            def conv(ot):
                for s in range(SPC):
                    for h0, nh in hblocks:
                        ps_t = psc.tile([128, HB, W], F32, tag="ps", name="ps")
                        n_mm = 0
                        total = IT * sum(
                            1 for dh, dw in taps if min(h0 + nh, H - dh) > max(h0, -dh)
                        )
                        for dh, dw in taps:
                            khkw = (dh + 1) * 3 + (dw + 1)
                            ho_s = max(h0, -dh)
                            ho_e = min(h0 + nh, H - dh)
                            if ho_e <= ho_s:
                                continue
                            nhh = ho_e - ho_s
                            hi_s = ho_s + dh
                            for it in range(IT):
                                off = khkw * 128
                                lhsT = cw_r[s, it, ot][:, off : off + 128]
                                rhs = x_sb[s, it][:, hi_s : hi_s + nhh, 1 + dw : 1 + dw + W]
                                out = ps_t[:, ho_s - h0 : ho_s - h0 + nhh, 0:W]
                                nc.tensor.matmul(
                                    out, lhsT, rhs,
                                    start=(n_mm == 0), stop=(n_mm == total - 1),
                                )
                                n_mm += 1
                        o_t = outp.tile([128, HB, W], F32, tag="out", name="o_t")
                        nc.scalar.activation(
                            o_t[:, :nh, :], ps_t[:, :nh, :], AF.Silu,
                            bias=bnb_sb[ot][:], scale=bns_sb[ot][:],
                        )
                        nc.gpsimd.dma_start(
                            y_d[s, ot, :, h0 : h0 + nh, :], o_t[:, :nh, :]
                        )

            def combine_dve(ot):
                cw_f32 = {
                    (s, it): cwp.tile(
                        [128, SLAB], F32, tag=f"cwf_{s}_{it}", name=f"cwf_{s}_{it}"
                    )
                    for s in range(SPC)
                    for it in range(IT)
                }
                for it in range(IT):
                    for e in range(E):
                        wt_t = slab_tiles[ot, it, e]
                        for s in range(SPC):
                            sc = r_bcast[s][:, e : e + 1]
                            if e == 0:
                                nc.vector.tensor_scalar_mul(
                                    cw_f32[s, it][:], wt_t[:], sc
                                )
                            elif e < E - 1:
                                nc.vector.scalar_tensor_tensor(
                                    cw_f32[s, it][:], wt_t[:], sc, cw_f32[s, it][:],
                                    ALU.mult, ALU.add,
                                )
                            else:
                                nc.vector.scalar_tensor_tensor(
                                    cw_r[s, it, ot][:], wt_t[:], sc, cw_f32[s, it][:],
                                    ALU.mult, ALU.add,
                                )

            combine_pe(0)
            combine_dve(1)
            conv(0)
            conv(1)

    nc.compile()
    return nc


def _get_program():
    if "nc" not in _PROGRAM_CACHE:
        _PROGRAM_CACHE["nc"] = _build_program()
    return _PROGRAM_CACHE["nc"]


def kernel(x, routing_w, routing_b, kernel_weights, bn_gamma, bn_beta, bn_mean, bn_var,
           _trace=False, _trace_kwargs=None):
    x = np.asarray(x, dtype=np.float32)
    routing_w = np.asarray(routing_w, dtype=np.float32)
    routing_b = np.asarray(routing_b, dtype=np.float32)
    kernel_weights = np.asarray(kernel_weights, dtype=np.float32)
    bn_gamma = np.asarray(bn_gamma, dtype=np.float32)
    bn_beta = np.asarray(bn_beta, dtype=np.float32)
    bn_mean = np.asarray(bn_mean, dtype=np.float32)
    bn_var = np.asarray(bn_var, dtype=np.float32)

    # wt[e, ot, it, i, khkw*128 + o_in] from kernel_weights[e, o, i, kh, kw]
    # o = ot*128 + o_in ; khkw = kh*3 + kw
    kw7 = kernel_weights.reshape(E, OT, 128, IT, 128, KS, KS)
    wt_host = np.ascontiguousarray(kw7.transpose(0, 1, 3, 4, 5, 6, 2)).reshape(
        E, OT, IT, 128, SLAB
    )
    rwt_host = np.ascontiguousarray(routing_w.T).reshape(IT, 128, E)
    rb_host = np.ascontiguousarray(routing_b).reshape(1, E)
    ident_host = np.eye(128, dtype=np.float32)
    inv = bn_gamma / np.sqrt(bn_var + BN_EPS)
    bns_host = np.ascontiguousarray(inv).reshape(OT, 128, 1)
    bnb_host = np.ascontiguousarray(bn_beta - bn_mean * inv).reshape(OT, 128, 1)

    x_pad = np.zeros((B, CIN, H, WP), dtype=np.float32)
    x_pad[:, :, :, 1 : 1 + W] = x
    in_maps = []
    for g in range(NCORES):
        xg = np.ascontiguousarray(
            x_pad[g * SPC : (g + 1) * SPC].reshape(SPC, IT, 128, H, WP)
        )
        in_maps.append(
            {
                "x": xg,
                "wt": wt_host,
                "rwt": rwt_host,
                "rb": rb_host,
                "ident": ident_host,
                "bns": bns_host,
                "bnb": bnb_host,
            }
        )

    nc = _get_program()
    res = run_bass_kernel_spmd(
        nc, in_maps, core_ids=list(range(NCORES)),
        trace=_trace, **(_trace_kwargs or {}),
    )
    _PROGRAM_CACHE["last_result"] = res

    out = np.empty((B, COUT, H, W), dtype=np.float32)
    for g in range(NCORES):
        yg = res.results[g]["y"]  # [SPC, OT, 128, H, W]
        out[g * SPC : (g + 1) * SPC] = yg.reshape(SPC, COUT, H, W)
    return out


# revision 8
# speedup vs baseline: 1.3801x; 1.0769x over previous
"""CondConv (MoE routed conv) Trainium2 Bass kernel.

Strategy (8 NeuronCores, data-parallel over batch, 2 samples/core):
  1. Routing on device: GAP via DVE reduce, linear via PE matmul, sigmoid on ACT.
  2. Per-sample combined conv weights cw[s] = sum_e r[s,e] * W[e] computed on
     the PE via a diagonal trick: cw_chunk = sum_e (r[s,e]*I).T @ W[e]_chunk,
     accumulated exactly in PSUM fp32, single float32r rounding on the ACT
     PSUM->SBUF copy.
  3. cw is laid out cout-half-major so the conv for output half ot=0 only
     needs the first half of the weight stream -> the DMA fill is halved.
  4. 3x3 conv as 18 accumulating PE matmuls per output tile (2 cin K-tiles x
     9 taps), float32r (1 cycle/row at N>=256, ~1.5e-4 rel err), zero padding
     via host-padded x width (58) and h-clipped access patterns + PSUM
     has_written semantics.
  5. BN (inference) + SiLU fused into one ACT activation per output tile
     (scale/bias per-partition vectors folded on host).
"""

import sys

sys.path.insert(0, "/opt/trn_rl_repo")

import numpy as np

import concourse.bass as bass  # noqa: F401
import concourse.mybir as mybir
import concourse.tile as tile
from concourse import bacc
from concourse.bass_utils import run_bass_kernel_spmd

F32 = mybir.dt.float32
F32R = mybir.dt.float32r
AF = mybir.ActivationFunctionType
ALU = mybir.AluOpType

B, CIN, H, W = 16, 256, 56, 56
E, COUT, KS = 8, 256, 3
NCORES = 8
SPC = B // NCORES  # samples per core
IT = CIN // 128  # cin partition tiles
OT = COUT // 128  # cout partition tiles
KHKW = KS * KS
HB = 8  # rows per h-block -> 7 blocks of 8, N = 8*56 = 448 (one PSUM bank)
WP = W + 2  # host-padded width (zero cols at w=0 and w=57)
PIX = H * W
BN_EPS = 1e-5
SLAB = KHKW * 128  # 1152 cw columns per (it, ot)
CHUNK = 384  # combine psum chunk (3 chunks per slab, >=256 keeps f32r rate)
NCH = SLAB // CHUNK

_PROGRAM_CACHE = {}


def _build_program():
    nc = bacc.Bacc("TRN2", target_bir_lowering=False, debug=False)

    x_d = nc.dram_tensor("x", [SPC, IT, 128, H, WP], F32R, kind="ExternalInput")
    # wt[e, ot, it, i, khkw*128 + o_in]  (slab-major for streaming by ot)
    wt_d = nc.dram_tensor("wt", [E, OT, IT, 128, SLAB], F32R, kind="ExternalInput")
    rwt_d = nc.dram_tensor("rwt", [IT, 128, E], F32, kind="ExternalInput")
    rb_d = nc.dram_tensor("rb", [1, E], F32, kind="ExternalInput")
    ident_d = nc.dram_tensor("ident", [128, 128], F32, kind="ExternalInput")
    bns_d = nc.dram_tensor("bns", [OT, 128, 1], F32, kind="ExternalInput")
    bnb_d = nc.dram_tensor("bnb", [OT, 128, 1], F32, kind="ExternalInput")
    y_d = nc.dram_tensor("y", [SPC, OT, 128, H, W], F32, kind="ExternalOutput")

    with tile.TileContext(nc) as tc:
        with (
            tc.tile_pool(name="xp", bufs=1) as xp,
            tc.tile_pool(name="cwp", bufs=1) as cwp,
            tc.tile_pool(name="wtp", bufs=10) as wtp,
            tc.tile_pool(name="outp", bufs=4) as outp,
            tc.tile_pool(name="smal", bufs=1) as smal,
            tc.tile_pool(name="psc", bufs=5, space="PSUM") as psc,
            tc.tile_pool(name="psk", bufs=2, space="PSUM") as psk,
            tc.tile_pool(name="pss", bufs=1, space="PSUM") as pss,
        ):
            # ---- input loads ----
            x_sb = {}
            for s in range(SPC):
                for it in range(IT):
                    t = xp.tile([128, H, WP], F32R, tag=f"x_{s}_{it}", name=f"x_{s}_{it}")
                    nc.sync.dma_start(t[:], x_d[s, it])
                    x_sb[s, it] = t

            rwt_sb = []
            for it in range(IT):
                t = smal.tile([128, E], F32, tag=f"rwt{it}", name=f"rwt{it}")
                nc.gpsimd.dma_start(t[:], rwt_d[it])
                rwt_sb.append(t)
            rb_sb = smal.tile([1, E], F32, tag="rb")
            nc.gpsimd.dma_start(rb_sb[:], rb_d[:])
            ident_sb = smal.tile([128, 128], F32, tag="ident")
            nc.gpsimd.dma_start(ident_sb[:], ident_d[:])
            bns_sb, bnb_sb = [], []
            for ot in range(OT):
                ts_ = smal.tile([128, 1], F32, tag=f"bns{ot}", name=f"bns{ot}")
                nc.gpsimd.dma_start(ts_[:], bns_d[ot])
                bns_sb.append(ts_)
                tb_ = smal.tile([128, 1], F32, tag=f"bnb{ot}", name=f"bnb{ot}")
                nc.gpsimd.dma_start(tb_[:], bnb_d[ot])
                bnb_sb.append(tb_)
            ones_sb = smal.tile([1, 128], F32, tag="ones")
            nc.vector.memset(ones_sb[:], 1.0)

            # ---- routing: r[s] = sigmoid(mean(x) @ rwT + rb) -> bcast [128, E] ----
            r_bcast = []
            for s in range(SPC):
                pooled = []
                for it in range(IT):
                    p = smal.tile([128, 1], F32, tag=f"pool{s}{it}", name=f"pool{s}{it}")
                    nc.vector.reduce_sum(
                        p[:],
                        x_sb[s, it][:].rearrange("p a b -> p (a b)"),
                        axis=mybir.AxisListType.X,
                    )
                    pooled.append(p)
                lg_ps = pss.tile([1, E], F32, tag="rps", name=f"lgps{s}")
                for it in range(IT):
                    nc.tensor.matmul(
                        lg_ps[:], pooled[it][:], rwt_sb[it][:],
                        start=(it == 0), stop=(it == IT - 1),
                    )
                zrow = smal.tile([1, E], F32, tag=f"z{s}", name=f"z{s}")
                nc.vector.scalar_tensor_tensor(
                    zrow[:], lg_ps[:], 1.0 / PIX, rb_sb[:], ALU.mult, ALU.add
                )
                rrow = smal.tile([1, E], F32, tag=f"r{s}", name=f"r{s}")
                nc.scalar.activation(rrow[:], zrow[:], AF.Sigmoid)
                rb_ps = pss.tile([128, E], F32, tag="rps", name=f"rbps{s}")
                nc.tensor.matmul(rb_ps[:], ones_sb[:], rrow[:], start=True, stop=True)
                rbc = smal.tile([128, E], F32, tag=f"rbc{s}", name=f"rbc{s}")
                nc.vector.tensor_copy(rbc[:], rb_ps[:])
                r_bcast.append(rbc)

            # diag[s,e] = r[s,e] * I, rounded to f32r by the ACT copy
            diag = {}
            for s in range(SPC):
                for e in range(E):
                    dt_ = smal.tile(
                        [128, 128], F32R, tag=f"diag{s}{e}", name=f"diag{s}{e}"
                    )
                    nc.scalar.activation(
                        dt_[:], ident_sb[:], AF.Copy, scale=r_bcast[s][:, e : e + 1]
                    )
                    diag[s, e] = dt_

            # persistent cw tiles (f32r), slab layout [i, khkw*128 + o_in]
            cw_r = {
                (s, it, ot): cwp.tile(
                    [128, SLAB], F32R,
                    tag=f"cwr_{s}_{it}_{ot}", name=f"cwr_{s}_{it}_{ot}",
                )
                for s in range(SPC)
                for it in range(IT)
                for ot in range(OT)
            }

            hblocks = [(h0, min(HB, H - h0)) for h0 in range(0, H, HB)]
            taps = [(0, 0)] + [
                (dh, dw) for dh in (-1, 0, 1) for dw in (-1, 0, 1) if (dh, dw) != (0, 0)
            ]

            # prefetch all wt slabs in consumption order on the sync ring
            # (x DMAs are ahead of them in the same FIFO; pool slots gate issue)
            slab_tiles = {}
            for ot in range(OT):
                for it in range(IT):
                    for e in range(E):
                        wt_t = wtp.tile(
                            [128, SLAB], F32R, tag="wt", name=f"wt{ot}{it}{e}"
                        )
                        nc.sync.dma_start(wt_t[:], wt_d[e, ot, it])
                        slab_tiles[ot, it, e] = wt_t

            def combine_pe_it(ot, it):
                # accumulate cw for one cin-tile on PE via diag trick
                slabs = [slab_tiles[ot, it, e] for e in range(E)]
                for c in range(NCH):
                    for s in range(SPC):
                        kps = psk.tile([128, CHUNK], F32, tag="kps", name="kps")
                        for e in range(E):
                            nc.tensor.matmul(
                                kps[:],
                                diag[s, e][:],
                                slabs[e][:, c * CHUNK : (c + 1) * CHUNK],
                                start=(e == 0),
                                stop=(e == E - 1),
                            )
                        nc.scalar.activation(
                            cw_r[s, it, ot][:, c * CHUNK : (c + 1) * CHUNK],
                            kps[:],
                            AF.Copy,
                        )

            def block_total(h0, nh):
                return IT * sum(
                    1 for dh, dw in taps if min(h0 + nh, H - dh) > max(h0, -dh)
                )

            def conv_block_taps(ot, s, h0, nh, ps_t, its, n_mm, total):
                # emit the tap matmuls of one conv block for the given cin
                # tiles; the PSUM accumulation group stays open until the
                # matmul carrying stop=True (n_mm == total-1)
                for it in its:
                    for dh, dw in taps:
                        khkw = (dh + 1) * 3 + (dw + 1)
                        ho_s = max(h0, -dh)
                        ho_e = min(h0 + nh, H - dh)
                        if ho_e <= ho_s:
                            continue
                        nhh = ho_e - ho_s
                        hi_s = ho_s + dh
                        off = khkw * 128
                        lhsT = cw_r[s, it, ot][:, off : off + 128]
                        rhs = x_sb[s, it][:, hi_s : hi_s + nhh, 1 + dw : 1 + dw + W]
                        out = ps_t[:, ho_s - h0 : ho_s - h0 + nhh, 0:W]
                        nc.tensor.matmul(
                            out, lhsT, rhs,
                            start=(n_mm == 0), stop=(n_mm == total - 1),
                        )
                        n_mm += 1
                return n_mm

            def conv_epilogue(ot, s, h0, nh, ps_t):
                o_t = outp.tile([128, HB, W], F32, tag="out", name="o_t")
                nc.scalar.activation(
                    o_t[:, :nh, :], ps_t[:, :nh, :], AF.Silu,
                    bias=bnb_sb[ot][:], scale=bns_sb[ot][:],
                )
                nc.gpsimd.dma_start(y_d[s, ot, :, h0 : h0 + nh, :], o_t[:, :nh, :])

            def conv(ot, skip=()):
                for s in range(SPC):
                    for h0, nh in hblocks:
                        if (s, h0) in skip:
                            continue
                        ps_t = psc.tile([128, HB, W], F32, tag="ps", name="ps")
                        total = block_total(h0, nh)
                        n_mm = conv_block_taps(ot, s, h0, nh, ps_t, range(IT), 0, total)
                        assert n_mm == total
                        conv_epilogue(ot, s, h0, nh, ps_t)

            def combine_dve(ot):
                cw_f32 = {
                    (s, it): cwp.tile(
                        [128, SLAB], F32, tag=f"cwf_{s}_{it}", name=f"cwf_{s}_{it}"
                    )
                    for s in range(SPC)
                    for it in range(IT)
                }
                for it in range(IT):
                    for e in range(E):
                        wt_t = slab_tiles[ot, it, e]
                        for s in range(SPC):
                            sc = r_bcast[s][:, e : e + 1]
                            if e == 0:
                                nc.vector.tensor_scalar_mul(
                                    cw_f32[s, it][:], wt_t[:], sc
                                )
                            elif e < E - 1:
                                nc.vector.scalar_tensor_tensor(
                                    cw_f32[s, it][:], wt_t[:], sc, cw_f32[s, it][:],
                                    ALU.mult, ALU.add,
                                )
                            else:
                                nc.vector.scalar_tensor_tensor(
                                    cw_r[s, it, ot][:], wt_t[:], sc, cw_f32[s, it][:],
                                    ALU.mult, ALU.add,
                                )

            # pipeline: combine it0 on PE -> phase-A (5 blocks, it0 taps only)
            # -> combine it1 on PE -> finish phase-A blocks -> rest of conv ot0
            # (combine for ot1 runs on the DVE, hidden under conv ot0)
            combine_pe_it(0, 0)
            NPA = 5  # open PSUM groups limited by psc pool size
            pa = []
            for h0, nh in hblocks[:NPA]:
                ps_t = psc.tile([128, HB, W], F32, tag="ps", name="ps")
                total = block_total(h0, nh)
                n_mm = conv_block_taps(0, 0, h0, nh, ps_t, [0], 0, total)
                pa.append((h0, nh, ps_t, n_mm, total))
            combine_pe_it(0, 1)
            combine_dve(1)
            for h0, nh, ps_t, n_mm, total in pa:
                n_mm = conv_block_taps(0, 0, h0, nh, ps_t, [1], n_mm, total)
                assert n_mm == total
                conv_epilogue(0, 0, h0, nh, ps_t)
            conv(0, skip={(0, h0) for h0, _, _, _, _ in pa})
            conv(1)

    nc.compile()
    return nc


def _get_program():
    if "nc" not in _PROGRAM_CACHE:
        _PROGRAM_CACHE["nc"] = _build_program()
    return _PROGRAM_CACHE["nc"]


def kernel(x, routing_w, routing_b, kernel_weights, bn_gamma, bn_beta, bn_mean, bn_var,
           _trace=False, _trace_kwargs=None):
    x = np.asarray(x, dtype=np.float32)
    routing_w = np.asarray(routing_w, dtype=np.float32)
    routing_b = np.asarray(routing_b, dtype=np.float32)
    kernel_weights = np.asarray(kernel_weights, dtype=np.float32)
    bn_gamma = np.asarray(bn_gamma, dtype=np.float32)
    bn_beta = np.asarray(bn_beta, dtype=np.float32)
    bn_mean = np.asarray(bn_mean, dtype=np.float32)
    bn_var = np.asarray(bn_var, dtype=np.float32)

    # wt[e, ot, it, i, khkw*128 + o_in] from kernel_weights[e, o, i, kh, kw]
    # o = ot*128 + o_in ; khkw = kh*3 + kw
    kw7 = kernel_weights.reshape(E, OT, 128, IT, 128, KS, KS)
    wt_host = np.ascontiguousarray(kw7.transpose(0, 1, 3, 4, 5, 6, 2)).reshape(
        E, OT, IT, 128, SLAB
    )
    rwt_host = np.ascontiguousarray(routing_w.T).reshape(IT, 128, E)
    rb_host = np.ascontiguousarray(routing_b).reshape(1, E)
    ident_host = np.eye(128, dtype=np.float32)
    inv = bn_gamma / np.sqrt(bn_var + BN_EPS)
    bns_host = np.ascontiguousarray(inv).reshape(OT, 128, 1)
    bnb_host = np.ascontiguousarray(bn_beta - bn_mean * inv).reshape(OT, 128, 1)

    x_pad = np.zeros((B, CIN, H, WP), dtype=np.float32)
    x_pad[:, :, :, 1 : 1 + W] = x
    in_maps = []
    for g in range(NCORES):
        xg = np.ascontiguousarray(
            x_pad[g * SPC : (g + 1) * SPC].reshape(SPC, IT, 128, H, WP)
        )
        in_maps.append(
            {
                "x": xg,
                "wt": wt_host,
                "rwt": rwt_host,
                "rb": rb_host,
                "ident": ident_host,
                "bns": bns_host,
                "bnb": bnb_host,
            }
        )

    nc = _get_program()
    res = run_bass_kernel_spmd(
        nc, in_maps, core_ids=list(range(NCORES)),
        trace=_trace, **(_trace_kwargs or {}),
    )
    _PROGRAM_CACHE["last_result"] = res

    out = np.empty((B, COUT, H, W), dtype=np.float32)
    for g in range(NCORES):
        yg = res.results[g]["y"]  # [SPC, OT, 128, H, W]
        out[g * SPC : (g + 1) * SPC] = yg.reshape(SPC, COUT, H, W)
    return out
